# revision 11
# baseline (speedup 1.0000x reference)
"""ALSH masked linear layer on 8 Trainium2 NeuronCores.

out = x @ (W * mask).T where mask selects W rows whose ALSH bucket matches the
bucket of the (normalized) mean query vector.

Sharding: W row-sharded across 8 cores (4096 rows each), x/a replicated,
out column-shards concatenated on the host. The only cross-core communication
is an AllReduce(max) of the per-shard max row-norm^2.

Compute strategy per core:
  phase 1: stream W once (row sumsq + row projection, bf16 copy to DRAM),
           stream x once (column sums on PE, bf16 transposed copy in SBUF)
  mask:    AllReduce(max norm^2) -> ALSH buckets -> row mask
  compact: per 512-column block, rank active rows (cumsum via triangular
           matmul), extract their indices (one-hot matmuls), dma_gather the
           active W rows transposed, matmul only those, then scatter columns
           back to dense positions with an exact 0/1 selection matmul.
"""

import sys
import types

for _p in ("/opt/trn_rl_repo", "/root/.axon_site"):
    if _p not in sys.path:
        sys.path.insert(0, _p)

import numpy as np
from contextlib import ExitStack

import concourse.bass as bass
import concourse.tile as tile
from concourse import bacc, mybir
from concourse import bass_isa

F32 = mybir.dt.float32
BF16 = mybir.dt.bfloat16
FP16 = mybir.dt.float16
I32 = mybir.dt.int32
I16 = mybir.dt.int16

M_AUG = 5
TABLE_SIZE = 64.0
R_BIN = 4.0
U_SCALE = 0.83

P = 128


def _install_ntff_shim():
    """Register the axon NTFF profile hook (antenv.axon_hooks is absent in
    this image); lets run_bass_kernel_spmd(trace=True) report exec_time_ns."""
    if "antenv.axon_hooks" in sys.modules:
        return
    try:
        from trn_agent_boot.trn_boot import _ntff_profile_via_ctypes

        hook = _ntff_profile_via_ctypes("/opt/axon/libaxon_pjrt.so")
        mod = types.ModuleType("antenv.axon_hooks")
        mod.get_axon_ntff_profile_hook = lambda: hook
        import antenv

        sys.modules["antenv.axon_hooks"] = mod
        antenv.axon_hooks = mod
    except Exception:
        pass


def build_program(B, D, Nl, n_cores, k_loc=256):
    """Build the per-core SPMD program. Nl = rows of W on this core.
    k_loc = compact slot capacity per 512-column output block."""
    assert B % P == 0 and D % P == 0 and Nl % P == 0
    NT = Nl // P            # W row-tiles
    BT = B // P             # x row-tiles
    DC = D // P             # contraction chunks of 128
    NFREE = min(512, Nl)    # output block width
    NB = Nl // NFREE        # output column blocks
    TPB = NFREE // P        # row-tiles per block
    AVE_F = min(512, D)
    AC = D // AVE_F         # ave psum chunks
    BF = min(512, B)
    BB = B // BF            # batch blocks for the compact matmul
    KL = k_loc
    MC = (KL + P - 1) // P  # slot chunks per block
    assert KL % P == 0 and NT <= P and KL <= 512

    nc = bacc.Bacc("TRN2", target_bir_lowering=False, debug=False,
                   num_devices=n_cores)

    x_h = nc.dram_tensor("x", [B, D], F32, kind="ExternalInput")
    w_h = nc.dram_tensor("W", [Nl, D], F32, kind="ExternalInput")
    a_h = nc.dram_tensor("a", [D + M_AUG], F32, kind="ExternalInput")
    out_h = nc.dram_tensor("out", [B, Nl], F32, kind="ExternalOutput")

    with tile.TileContext(nc) as tc, ExitStack() as ctx:
        const = ctx.enter_context(tc.tile_pool(name="const", bufs=1))
        wpool = ctx.enter_context(tc.tile_pool(name="wpool", bufs=3))
        wbpool = ctx.enter_context(tc.tile_pool(name="wbpool", bufs=2))
        junk = ctx.enter_context(tc.tile_pool(name="junk", bufs=1))
        wctpool = ctx.enter_context(tc.tile_pool(name="wctpool", bufs=2))
        ctpool = ctx.enter_context(tc.tile_pool(name="ctpool", bufs=2))
        selpool = ctx.enter_context(tc.tile_pool(name="selpool", bufs=2))
        outpool = ctx.enter_context(tc.tile_pool(name="outpool", bufs=3))
        small = ctx.enter_context(tc.tile_pool(name="small", bufs=1))
        psum_misc = ctx.enter_context(
            tc.tile_pool(name="psum_misc", bufs=3, space="PSUM"))
        psum_mm = ctx.enter_context(
            tc.tile_pool(name="psum_mm", bufs=2, space="PSUM"))
        psum_sc = ctx.enter_context(
            tc.tile_pool(name="psum_sc", bufs=2, space="PSUM"))
        dram = ctx.enter_context(tc.tile_pool(name="dram", bufs=1, space="DRAM"))

        # ---- constants ----------------------------------------------------
        a_row = const.tile([1, D], F32)
        nc.sync.dma_start(a_row[:], a_h.ap()[0:D])
        aug_row = const.tile([1, M_AUG], F32)
        nc.sync.dma_start(aug_row[:], a_h.ap()[D:D + M_AUG])
        a_rep = const.tile([P, D], F32)
        nc.gpsimd.partition_broadcast(a_rep[:], a_row[:])
        aug_rep = const.tile([P, M_AUG], F32)
        nc.gpsimd.partition_broadcast(aug_rep[:], aug_row[:])
        ones_col = const.tile([P, 1], F32)
        nc.vector.memset(ones_col[:], 1.0)

        # iota-derived constants
        iop_i = junk.tile([P, 1], I32, tag="j2")
        nc.gpsimd.iota(iop_i[:], pattern=[[0, 1]], base=0, channel_multiplier=1)
        iop_f = const.tile([P, 1], F32)
        nc.vector.tensor_copy(out=iop_f[:], in_=iop_i[:])
        iotam = []
        for m in range(MC):
            im = const.tile([P, 1], F32, name=f"iotam{m}")
            nc.vector.tensor_scalar_add(im[:], iop_f[:], float(m * P))
            iotam.append(im)
        # upper-triangular ones (U[k, m] = 1 for m >= k) for partition cumsum
        iof_i = junk.tile([P, P], I32, tag="j2")
        nc.gpsimd.iota(iof_i[:], pattern=[[1, P]], base=0, channel_multiplier=0)
        iof_f = const.tile([P, P], F32)
        nc.vector.tensor_copy(out=iof_f[:], in_=iof_i[:])
        u_tri = const.tile([P, P], BF16)
        nc.vector.tensor_scalar(
            out=u_tri[:], in0=iof_f[:], scalar1=iop_f[:], scalar2=None,
            op0=mybir.AluOpType.is_ge)
        # iota row 0..KL-1 on every partition (slot ids)
        iok_i = junk.tile([P, KL], I32, tag="j2")
        nc.gpsimd.iota(iok_i[:], pattern=[[1, KL]], base=0, channel_multiplier=0)
        iok_f = const.tile([P, KL], F32)
        nc.vector.tensor_copy(out=iok_f[:], in_=iok_i[:])
        # per-row n values, split exactly into bf16-safe parts: n = 128*t + p
        pvals = const.tile([P, 1], BF16)
        nc.vector.tensor_copy(out=pvals[:], in_=iop_f[:])
        tv_i = junk.tile([P, NT], I32, tag="j2")
        nc.gpsimd.iota(tv_i[:], pattern=[[1, NT]], base=0, channel_multiplier=0)
        tvals = const.tile([P, NT], BF16)
        nc.vector.tensor_copy(out=tvals[:], in_=tv_i[:])

        # persistent per-row stats ([p, t] layout; row n = t*128 + p)
        sumsq = const.tile([P, NT], F32)
        projr = const.tile([P, NT], F32)
        xT = const.tile([P, DC, B], BF16)      # transposed x, bf16
        selT = const.tile([P, NT, KL], BF16)   # one-hot rank indicators
        wbf_d = dram.tile([Nl, D], BF16)       # bf16 copy of W shard

        # ---- phase 1a: stream W; row sumsq + raw projection + bf16 copy ---
        for t in range(NT):
            wt = wpool.tile([P, D], F32, tag="wt")
            nc.sync.dma_start(wt[:], w_h.ap()[t * P:(t + 1) * P, :])
            j1 = junk.tile([P, D], F32, tag="j1")
            nc.scalar.activation(
                j1[:], wt[:], mybir.ActivationFunctionType.Square,
                accum_out=sumsq[:, t:t + 1])
            j2 = junk.tile([P, D], F32, tag="j2")
            nc.vector.scalar_tensor_tensor(
                out=j2[:], in0=wt[:], scalar=1.0, in1=a_rep[:],
                op0=mybir.AluOpType.mult, op1=mybir.AluOpType.mult,
                accum_out=projr[:, t:t + 1])
            wb = wbpool.tile([P, D], BF16, tag="wb")
            nc.vector.tensor_copy(out=wb[:], in_=wt[:])
            nc.scalar.dma_start(wbf_d[t * P:(t + 1) * P, :], wb[:])

        # ---- phase 1b: stream x; column sums (PE) + bf16 transposed copy --
        colsum = small.tile([1, D], F32)
        nc.vector.memset(colsum[:], 0.0)
        for bt in range(BT):
            xt = wpool.tile([P, D], F32, tag="wt")
            nc.sync.dma_start(xt[:], x_h.ap()[bt * P:(bt + 1) * P, :])
            for c in range(AC):
                pa = psum_misc.tile([1, AVE_F], F32, tag="misc")
                nc.tensor.matmul(
                    pa[:], lhsT=ones_col[:],
                    rhs=xt[:, c * AVE_F:(c + 1) * AVE_F],
                    start=True, stop=True)
                nc.vector.tensor_add(
                    colsum[0:1, c * AVE_F:(c + 1) * AVE_F],
                    colsum[0:1, c * AVE_F:(c + 1) * AVE_F], pa[:])
            xb = wbpool.tile([P, D], BF16, tag="wb")
            nc.vector.tensor_copy(out=xb[:], in_=xt[:])
            nc.sync.dma_start(
                xT[:, :, bt * P:(bt + 1) * P], xb[:], transpose=True)

        # ---- query bucket (identical on every core) -----------------------
        qj1 = junk.tile([1, D], F32, tag="j1")
        ss = small.tile([1, 1], F32)     # sum(colsum^2)
        nc.scalar.activation(
            qj1[:], colsum[:], mybir.ActivationFunctionType.Square,
            accum_out=ss[:])
        qj2 = junk.tile([1, D], F32, tag="j2")
        s1 = small.tile([1, 1], F32)     # sum(colsum * a)
        nc.vector.scalar_tensor_tensor(
            out=qj2[:], in0=colsum[:], scalar=1.0, in1=a_row[:],
            op0=mybir.AluOpType.mult, op1=mybir.AluOpType.mult,
            accum_out=s1[:])
        augsum = small.tile([1, 1], F32)
        nc.vector.reduce_sum(augsum[:], aug_row[:], axis=mybir.AxisListType.X)

        ssr = small.tile([1, 1], F32)
        nc.scalar.sqrt(ssr[:], ss[:])            # |colsum|
        ssi = small.tile([1, 1], F32)
        nc.vector.reciprocal(ssi[:], ssr[:])
        qp = small.tile([1, 1], F32)
        nc.vector.tensor_mul(qp[:], s1[:], ssi[:])      # q . a[:D]
        nc.vector.scalar_tensor_tensor(
            out=qp[:], in0=augsum[:], scalar=0.5, in1=qp[:],
            op0=mybir.AluOpType.mult, op1=mybir.AluOpType.add)
        qdiv = small.tile([1, 1], F32)
        nc.vector.tensor_scalar_mul(qdiv[:], qp[:], 1.0 / R_BIN)

        def floor_(dst_pool, src, shape, tag):
            ti = dst_pool.tile(shape, I32, tag=tag + "_i", name=tag + "_i")
            nc.vector.tensor_copy(out=ti[:], in_=src[:])
            tf = dst_pool.tile(shape, F32, tag=tag + "_f", name=tag + "_f")
            nc.vector.tensor_copy(out=tf[:], in_=ti[:])
            gt = dst_pool.tile(shape, F32, tag=tag + "_g", name=tag + "_g")
            nc.vector.tensor_tensor(
                out=gt[:], in0=tf[:], in1=src[:], op=mybir.AluOpType.is_gt)
            h = dst_pool.tile(shape, F32, tag=tag + "_h", name=tag + "_h")
            nc.vector.tensor_sub(h[:], tf[:], gt[:])
            return h

        def bucket_(dst_pool, src, shape, tag):
            # abs(mod(floor(src), TABLE_SIZE)) for values in (-TS, TS)
            h = floor_(dst_pool, src, shape, tag)
            neg = dst_pool.tile(shape, F32, tag=tag + "_n", name=tag + "_n")
            nc.vector.tensor_scalar(
                out=neg[:], in0=h[:], scalar1=0.0, scalar2=None,
                op0=mybir.AluOpType.is_lt)
            nc.vector.scalar_tensor_tensor(
                out=h[:], in0=neg[:], scalar=TABLE_SIZE, in1=h[:],
                op0=mybir.AluOpType.mult, op1=mybir.AluOpType.add)
            return h

        qb = bucket_(small, qdiv, [1, 1], "qb")
        qb_rep = small.tile([P, 1], F32)
        nc.gpsimd.partition_broadcast(qb_rep[:], qb[:])

        # ---- global max of sumsq (AllReduce over cores) -------------------
        lmax = small.tile([P, 1], F32)
        nc.vector.reduce_max(lmax[:], sumsq[:], axis=mybir.AxisListType.X)
        amax = small.tile([P, 1], F32)
        nc.gpsimd.partition_all_reduce(
            amax[:], lmax[:], channels=P, reduce_op=bass_isa.ReduceOp.max)
        mx_in = dram.tile([1, 1], F32)
        mx_out = dram.tile([1, 1], F32)
        nc.sync.dma_start(mx_in[:], amax[0:1, :])
        nc.gpsimd.collective_compute(
            "AllReduce", mybir.AluOpType.max,
            replica_groups=[list(range(n_cores))],
            ins=[mx_in.opt()], outs=[mx_out.opt()])
        gsb = small.tile([1, 1], F32)
        nc.sync.dma_start(gsb[:], mx_out[:])
        gmax = small.tile([P, 1], F32)
        nc.gpsimd.partition_broadcast(gmax[:], gsb[:])

        # scale s = U / sqrt(gmax), replicated on all partitions
        sq = small.tile([P, 1], F32)
        nc.scalar.sqrt(sq[:], gmax[:])
        sinv = small.tile([P, 1], F32)
        nc.vector.reciprocal(sinv[:], sq[:])
        s_rep = small.tile([P, 1], F32)
        nc.vector.tensor_scalar_mul(s_rep[:], sinv[:], U_SCALE)
        s2_rep = small.tile([P, 1], F32)
        nc.vector.tensor_mul(s2_rep[:], s_rep[:], s_rep[:])

        # ---- per-row bucket + mask ([P, NT]) ------------------------------
        n2 = small.tile([P, NT], F32)
        nc.vector.tensor_scalar(
            out=n2[:], in0=sumsq[:], scalar1=s2_rep[:], scalar2=None,
            op0=mybir.AluOpType.mult)
        proj = small.tile([P, NT], F32)
        nc.vector.tensor_scalar(
            out=proj[:], in0=projr[:], scalar1=s_rep[:], scalar2=None,
            op0=mybir.AluOpType.mult)
        pw = small.tile([P, NT], F32)
        nc.vector.tensor_copy(out=pw[:], in_=n2[:])
        for i in range(M_AUG):
            nc.vector.scalar_tensor_tensor(
                out=proj[:], in0=pw[:], scalar=aug_rep[:, i:i + 1], in1=proj[:],
                op0=mybir.AluOpType.mult, op1=mybir.AluOpType.add)
            if i < M_AUG - 1:
                nc.vector.tensor_mul(pw[:], pw[:], pw[:])
        pdiv = small.tile([P, NT], F32)
        nc.vector.tensor_scalar_mul(pdiv[:], proj[:], 1.0 / R_BIN)
        rb = bucket_(small, pdiv, [P, NT], "rb")
        maskf = small.tile([P, NT], F32)
        nc.vector.tensor_scalar(
            out=maskf[:], in0=rb[:], scalar1=qb_rep[:], scalar2=None,
            op0=mybir.AluOpType.is_equal)

        # ---- compaction: per-block ranks ----------------------------------
        maskb = small.tile([P, NT], BF16)
        nc.vector.tensor_copy(out=maskb[:], in_=maskf[:])
        cc_ps = psum_misc.tile([P, NT], F32, tag="misc")
        nc.tensor.matmul(cc_ps[:], lhsT=u_tri[:], rhs=maskb[:],
                         start=True, stop=True)
        cc = small.tile([P, NT], F32)
        nc.vector.tensor_copy(out=cc[:], in_=cc_ps[:])
        csrow = small.tile([1, NT], F32)
        nc.sync.dma_start(csrow[:], cc[P - 1:P, :])
        co = small.tile([1, NT], F32)
        nc.vector.memset(co[:], 0.0)
        co3 = co[0:1, :].rearrange("o (b t) -> o b t", t=TPB)
        cs3 = csrow[0:1, :].rearrange("o (b t) -> o b t", t=TPB)
        for k in range(1, TPB):
            nc.vector.tensor_add(
                co3[:, :, k:TPB], co3[:, :, k:TPB], cs3[:, :, 0:TPB - k])
        co_rep = small.tile([P, NT], F32)
        nc.gpsimd.partition_broadcast(co_rep[:], co[:])
        # in-block rank of each row (exclusive cumsum), -1 for inactive rows
        rank = small.tile([P, NT], F32)
        nc.vector.tensor_sub(rank[:], cc[:], maskf[:])
        nc.vector.tensor_add(rank[:], rank[:], co_rep[:])
        rank1 = small.tile([P, NT], F32)
        nc.vector.tensor_scalar_add(rank1[:], rank[:], 1.0)
        renc = small.tile([P, NT], F32)
        nc.vector.tensor_mul(renc[:], rank1[:], maskf[:])
        nc.vector.tensor_scalar_add(renc[:], renc[:], -1.0)

        # one-hot rank indicators per row tile: selT[p, t, k] = (renc == k)
        for t in range(NT):
            nc.vector.tensor_scalar(
                out=selT[:, t, :], in0=iok_f[:], scalar1=renc[:, t:t + 1],
                scalar2=None, op0=mybir.AluOpType.is_equal)

        # rank row view (transposed, fp16 keeps ints <= 2048 exact)
        rpad = small.tile([P, P], FP16)
        nc.vector.memset(rpad[:], -1.0)
        nc.vector.tensor_copy(out=rpad[:, 0:NT], in_=renc[:])
        rT = small.tile([P, P], FP16)
        nc.sync.dma_start(rT[:], rpad[:], transpose=True)
        rank_d = dram.tile([NT, P], FP16)
        nc.sync.dma_start(rank_d[:], rT[0:NT, :])

        # per-block compact indices: idx[s] = n of the row with rank s
        idx_d = dram.tile([NB, KL], I16)
        for j in range(NB):
            ip1 = psum_misc.tile([1, KL], F32, tag="misc", name=f"ip1_{j}")
            ip2 = psum_misc.tile([1, KL], F32, tag="misc", name=f"ip2_{j}")
            for ti in range(TPB):
                t = j * TPB + ti
                nc.tensor.matmul(ip1[:], lhsT=pvals[:], rhs=selT[:, t, :],
                                 start=(ti == 0), stop=(ti == TPB - 1))
                nc.tensor.matmul(ip2[:], lhsT=tvals[:, t:t + 1],
                                 rhs=selT[:, t, :],
                                 start=(ti == 0), stop=(ti == TPB - 1))
            ip1s = small.tile([1, KL], F32, tag="ip1s", name=f"ip1s{j}")
            nc.vector.tensor_copy(out=ip1s[:], in_=ip1[:])
            idxf = small.tile([1, KL], F32, tag="idxf", name=f"idxf{j}")
            nc.vector.scalar_tensor_tensor(
                out=idxf[:], in0=ip2[:], scalar=float(P), in1=ip1s[:],
                op0=mybir.AluOpType.mult, op1=mybir.AluOpType.add)
            idx16 = small.tile([1, KL], I16, tag="idx16", name=f"idx16_{j}")
            nc.vector.tensor_copy(out=idx16[:], in_=idxf[:])
            nc.sync.dma_start(idx_d[j:j + 1, :], idx16[:])
        idxw = const.tile([P, NB, KL // 16], I16)
        for g in range(8):
            nc.sync.dma_start(
                idxw[g * 16:(g + 1) * 16, :, :],
                idx_d[:].rearrange("j (c p) -> p j c", p=16))

        # ---- phase 2: per-block gather + compact matmul + select-scatter --
        for j in range(NB):
            wct = wctpool.tile([P, DC, KL], BF16, tag="wct", name=f"wct{j}")
            nc.gpsimd.dma_gather(
                out_ap=wct[:], in_ap=wbf_d[:], idxs_ap=idxw[:, j, :],
                num_idxs=KL, num_idxs_reg=KL, elem_size=D, transpose=True)

            # compactT[slot, b] = sum_d WT[d, idx[slot]] * xT[d, b]
            ct = ctpool.tile([P, MC, B], BF16, tag="ct", name=f"ct{j}")
            for m in range(MC):
                for bb in range(BB):
                    pm = psum_mm.tile([P, BF], F32, tag="mm",
                                      name=f"pm{j}_{m}_{bb}")
                    for dc in range(DC):
                        nc.tensor.matmul(
                            pm[:], lhsT=wct[:, dc, m * P:(m + 1) * P],
                            rhs=xT[:, dc, bb * BF:(bb + 1) * BF],
                            start=(dc == 0), stop=(dc == DC - 1))
                    nc.vector.tensor_copy(
                        out=ct[:, m, bb * BF:(bb + 1) * BF], in_=pm[:])

            # selection matrix for this block: sel[s, n] = (rank(n) == s)
            rrow = selpool.tile([1, NFREE], FP16, tag="rrow", name=f"rr{j}")
            nc.sync.dma_start(
                rrow[:],
                rank_d[j * TPB:(j + 1) * TPB, :].rearrange("a b -> (a b)"))
            rrep = selpool.tile([P, NFREE], FP16, tag="rrep", name=f"rrep{j}")
            nc.gpsimd.partition_broadcast(rrep[:], rrow[:])
            sel = selpool.tile([P, MC, NFREE], BF16, tag="sel", name=f"sel{j}")
            for m in range(MC):
                nc.vector.tensor_scalar(
                    out=sel[:, m, :], in0=rrep[:], scalar1=iotam[m],
                    scalar2=None, op0=mybir.AluOpType.is_equal)

            # dense output block via selection matmul (exact zeros elsewhere)
            for bt in range(BT):
                po = psum_sc.tile([P, NFREE], F32, tag="sc",
                                  name=f"po{j}_{bt}")
                for m in range(MC):
                    nc.tensor.matmul(
                        po[:], lhsT=ct[:, m, bt * P:(bt + 1) * P],
                        rhs=sel[:, m, :],
                        start=(m == 0), stop=(m == MC - 1))
                ob = outpool.tile([P, NFREE], F32, tag="ob",
                                  name=f"ob{j}_{bt}")
                nc.vector.tensor_copy(out=ob[:], in_=po[:])
                nc.scalar.dma_start(
                    out_h.ap()[bt * P:(bt + 1) * P,
                               j * NFREE:(j + 1) * NFREE], ob[:])

    nc.compile()
    return nc


_CACHE = {}
LAST_RESULTS = None


def _get_program(B, D, Nl, n_cores):
    key = (B, D, Nl, n_cores)
    if key not in _CACHE:
        _CACHE[key] = build_program(B, D, Nl, n_cores)
    return _CACHE[key]


def kernel(x, W, a, mode=None, trace=False):
    global LAST_RESULTS
    _install_ntff_shim()
    from concourse.bass_utils import run_bass_kernel_spmd

    x = np.ascontiguousarray(np.asarray(x, dtype=np.float32))
    W = np.ascontiguousarray(np.asarray(W, dtype=np.float32))
    a = np.ascontiguousarray(np.asarray(a, dtype=np.float32))
    B, D = x.shape
    N = W.shape[0]
    n_cores = 8
    Nl = N // n_cores
    nc = _get_program(B, D, Nl, n_cores)

    in_maps = [
        {"x": x, "W": W[i * Nl:(i + 1) * Nl], "a": a} for i in range(n_cores)
    ]
    res = run_bass_kernel_spmd(
        nc, in_maps, core_ids=list(range(n_cores)), trace=trace)
    LAST_RESULTS = res
    out = np.concatenate([res.results[i]["out"] for i in range(n_cores)], axis=1)
    return out.astype(np.float32)


# revision 12
# speedup vs baseline: 1.0607x; 1.0607x over previous
"""ALSH masked linear layer on 8 Trainium2 NeuronCores.

out = x @ (W * mask).T where mask selects W rows whose ALSH bucket matches the
bucket of the (normalized) mean query vector.

Sharding: W row-sharded across 8 cores (4096 rows each), x/a replicated,
out column-shards concatenated on the host. The only cross-core communication
is an AllReduce(max) of the per-shard max row-norm^2.

Compute strategy per core:
  phase 1: stream W once (row sumsq + row projection, bf16 copy to DRAM),
           stream x once (column sums on PE, bf16 transposed copy in SBUF)
  mask:    AllReduce(max norm^2) -> ALSH buckets -> row mask
  compact: per 512-column block, rank active rows (cumsum via triangular
           matmul), extract their indices (one-hot matmuls), dma_gather the
           active W rows transposed, matmul only those, then scatter columns
           back to dense positions with an exact 0/1 selection matmul.
"""

import sys
import types

for _p in ("/opt/trn_rl_repo", "/root/.axon_site"):
    if _p not in sys.path:
        sys.path.insert(0, _p)

import numpy as np
from contextlib import ExitStack

import concourse.bass as bass
import concourse.tile as tile
from concourse import bacc, mybir
from concourse import bass_isa

F32 = mybir.dt.float32
BF16 = mybir.dt.bfloat16
FP16 = mybir.dt.float16
I32 = mybir.dt.int32
I16 = mybir.dt.int16

M_AUG = 5
TABLE_SIZE = 64.0
R_BIN = 4.0
U_SCALE = 0.83

P = 128


def _install_ntff_shim():
    """Register the axon NTFF profile hook (antenv.axon_hooks is absent in
    this image); lets run_bass_kernel_spmd(trace=True) report exec_time_ns."""
    if "antenv.axon_hooks" in sys.modules:
        return
    try:
        from trn_agent_boot.trn_boot import _ntff_profile_via_ctypes

        hook = _ntff_profile_via_ctypes("/opt/axon/libaxon_pjrt.so")
        mod = types.ModuleType("antenv.axon_hooks")
        mod.get_axon_ntff_profile_hook = lambda: hook
        import antenv

        sys.modules["antenv.axon_hooks"] = mod
        antenv.axon_hooks = mod
    except Exception:
        pass


def build_program(B, D, Nl, n_cores, k_loc=256):
    """Build the per-core SPMD program. Nl = rows of W on this core.
    k_loc = compact slot capacity per 512-column output block."""
    assert B % P == 0 and D % P == 0 and Nl % P == 0
    NT = Nl // P            # W row-tiles
    BT = B // P             # x row-tiles
    DC = D // P             # contraction chunks of 128
    NFREE = min(512, Nl)    # output block width
    NB = Nl // NFREE        # output column blocks
    TPB = NFREE // P        # row-tiles per block
    AVE_F = min(512, D)
    AC = D // AVE_F         # ave psum chunks
    BF = min(512, B)
    BB = B // BF            # batch blocks for the compact matmul
    KL = k_loc
    MC = (KL + P - 1) // P  # slot chunks per block
    assert KL % P == 0 and NT <= P and KL <= 512

    nc = bacc.Bacc("TRN2", target_bir_lowering=False, debug=False,
                   num_devices=n_cores)

    x_h = nc.dram_tensor("x", [B, D], F32, kind="ExternalInput")
    w_h = nc.dram_tensor("W", [Nl, D], F32, kind="ExternalInput")
    a_h = nc.dram_tensor("a", [D + M_AUG], F32, kind="ExternalInput")
    out_h = nc.dram_tensor("out", [B, Nl], F32, kind="ExternalOutput")

    with tile.TileContext(nc) as tc, ExitStack() as ctx:
        const = ctx.enter_context(tc.tile_pool(name="const", bufs=1))
        wpool = ctx.enter_context(tc.tile_pool(name="wpool", bufs=3))
        wbpool = ctx.enter_context(tc.tile_pool(name="wbpool", bufs=2))
        junk = ctx.enter_context(tc.tile_pool(name="junk", bufs=1))
        wctpool = ctx.enter_context(tc.tile_pool(name="wctpool", bufs=2))
        ctpool = ctx.enter_context(tc.tile_pool(name="ctpool", bufs=2))
        selpool = ctx.enter_context(tc.tile_pool(name="selpool", bufs=2))
        outpool = ctx.enter_context(tc.tile_pool(name="outpool", bufs=3))
        small = ctx.enter_context(tc.tile_pool(name="small", bufs=1))
        psum_misc = ctx.enter_context(
            tc.tile_pool(name="psum_misc", bufs=3, space="PSUM"))
        psum_mm = ctx.enter_context(
            tc.tile_pool(name="psum_mm", bufs=3, space="PSUM"))
        psum_sc = ctx.enter_context(
            tc.tile_pool(name="psum_sc", bufs=2, space="PSUM"))
        dram = ctx.enter_context(tc.tile_pool(name="dram", bufs=1, space="DRAM"))

        # ---- constants ----------------------------------------------------
        a_row = const.tile([1, D], F32)
        nc.sync.dma_start(a_row[:], a_h.ap()[0:D])
        aug_row = const.tile([1, M_AUG], F32)
        nc.sync.dma_start(aug_row[:], a_h.ap()[D:D + M_AUG])
        a_rep = const.tile([P, D], F32)
        nc.gpsimd.partition_broadcast(a_rep[:], a_row[:])
        aug_rep = const.tile([P, M_AUG], F32)
        nc.gpsimd.partition_broadcast(aug_rep[:], aug_row[:])
        ones_col = const.tile([P, 1], F32)
        nc.vector.memset(ones_col[:], 1.0)

        # iota-derived constants
        iop_i = junk.tile([P, 1], I32, tag="j2")
        nc.gpsimd.iota(iop_i[:], pattern=[[0, 1]], base=0, channel_multiplier=1)
        iop_f = const.tile([P, 1], F32)
        nc.vector.tensor_copy(out=iop_f[:], in_=iop_i[:])
        iotam = []
        for m in range(MC):
            im = const.tile([P, 1], F32, name=f"iotam{m}")
            nc.vector.tensor_scalar_add(im[:], iop_f[:], float(m * P))
            iotam.append(im)
        # upper-triangular ones (U[k, m] = 1 for m >= k) for partition cumsum
        iof_i = junk.tile([P, P], I32, tag="j2")
        nc.gpsimd.iota(iof_i[:], pattern=[[1, P]], base=0, channel_multiplier=0)
        iof_f = const.tile([P, P], F32)
        nc.vector.tensor_copy(out=iof_f[:], in_=iof_i[:])
        u_tri = const.tile([P, P], BF16)
        nc.vector.tensor_scalar(
            out=u_tri[:], in0=iof_f[:], scalar1=iop_f[:], scalar2=None,
            op0=mybir.AluOpType.is_ge)
        # iota row 0..KL-1 on every partition (slot ids)
        iok_i = junk.tile([P, KL], I32, tag="j2")
        nc.gpsimd.iota(iok_i[:], pattern=[[1, KL]], base=0, channel_multiplier=0)
        iok_f = const.tile([P, KL], F32)
        nc.vector.tensor_copy(out=iok_f[:], in_=iok_i[:])
        # per-row n values, split exactly into bf16-safe parts: n = 128*t + p
        pvals = const.tile([P, 1], BF16)
        nc.vector.tensor_copy(out=pvals[:], in_=iop_f[:])
        tv_i = junk.tile([P, NT], I32, tag="j2")
        nc.gpsimd.iota(tv_i[:], pattern=[[1, NT]], base=0, channel_multiplier=0)
        tvals = const.tile([P, NT], BF16)
        nc.vector.tensor_copy(out=tvals[:], in_=tv_i[:])

        # persistent per-row stats ([p, t] layout; row n = t*128 + p)
        sumsq = const.tile([P, NT], F32)
        projr = const.tile([P, NT], F32)
        xT = const.tile([P, DC, B], BF16)      # transposed x, bf16
        selT = const.tile([P, NT, KL], BF16)   # one-hot rank indicators
        wbf_d = dram.tile([Nl, D], BF16)       # bf16 copy of W shard

        # ---- phase 1a: stream W; row sumsq + raw projection + bf16 copy ---
        for t in range(NT):
            wt = wpool.tile([P, D], F32, tag="wt")
            nc.sync.dma_start(wt[:], w_h.ap()[t * P:(t + 1) * P, :])
            j1 = junk.tile([P, D], F32, tag="j1")
            nc.scalar.activation(
                j1[:], wt[:], mybir.ActivationFunctionType.Square,
                accum_out=sumsq[:, t:t + 1])
            j2 = junk.tile([P, D], F32, tag="j2")
            nc.vector.scalar_tensor_tensor(
                out=j2[:], in0=wt[:], scalar=1.0, in1=a_rep[:],
                op0=mybir.AluOpType.mult, op1=mybir.AluOpType.mult,
                accum_out=projr[:, t:t + 1])
            wb = wbpool.tile([P, D], BF16, tag="wb")
            nc.vector.tensor_copy(out=wb[:], in_=wt[:])
            nc.scalar.dma_start(wbf_d[t * P:(t + 1) * P, :], wb[:])

        # ---- global max of sumsq (AllReduce over cores) -------------------
        lmax = small.tile([P, 1], F32)
        nc.vector.reduce_max(lmax[:], sumsq[:], axis=mybir.AxisListType.X)
        amax = small.tile([P, 1], F32)
        nc.gpsimd.partition_all_reduce(
            amax[:], lmax[:], channels=P, reduce_op=bass_isa.ReduceOp.max)
        mx_in = dram.tile([1, 1], F32)
        mx_out = dram.tile([1, 1], F32)
        nc.sync.dma_start(mx_in[:], amax[0:1, :])
        nc.gpsimd.collective_compute(
            "AllReduce", mybir.AluOpType.max,
            replica_groups=[list(range(n_cores))],
            ins=[mx_in.opt()], outs=[mx_out.opt()])
        gsb = small.tile([1, 1], F32)
        nc.sync.dma_start(gsb[:], mx_out[:])
        gmax = small.tile([P, 1], F32)
        nc.gpsimd.partition_broadcast(gmax[:], gsb[:])

        # ---- phase 1b: stream x; column sums (PE) + bf16 transposed copy --
        colsum = small.tile([1, D], F32)
        nc.vector.memset(colsum[:], 0.0)
        for bt in range(BT):
            xt = wpool.tile([P, D], F32, tag="wt")
            nc.sync.dma_start(xt[:], x_h.ap()[bt * P:(bt + 1) * P, :])
            for c in range(AC):
                pa = psum_misc.tile([1, AVE_F], F32, tag="misc")
                nc.tensor.matmul(
                    pa[:], lhsT=ones_col[:],
                    rhs=xt[:, c * AVE_F:(c + 1) * AVE_F],
                    start=True, stop=True)
                nc.vector.tensor_add(
                    colsum[0:1, c * AVE_F:(c + 1) * AVE_F],
                    colsum[0:1, c * AVE_F:(c + 1) * AVE_F], pa[:])
            xb = wbpool.tile([P, D], BF16, tag="wb")
            nc.vector.tensor_copy(out=xb[:], in_=xt[:])
            nc.sync.dma_start(
                xT[:, :, bt * P:(bt + 1) * P], xb[:], transpose=True)

        # ---- query bucket (identical on every core) -----------------------
        qj1 = junk.tile([1, D], F32, tag="j1")
        ss = small.tile([1, 1], F32)     # sum(colsum^2)
        nc.scalar.activation(
            qj1[:], colsum[:], mybir.ActivationFunctionType.Square,
            accum_out=ss[:])
        qj2 = junk.tile([1, D], F32, tag="j2")
        s1 = small.tile([1, 1], F32)     # sum(colsum * a)
        nc.vector.scalar_tensor_tensor(
            out=qj2[:], in0=colsum[:], scalar=1.0, in1=a_row[:],
            op0=mybir.AluOpType.mult, op1=mybir.AluOpType.mult,
            accum_out=s1[:])
        augsum = small.tile([1, 1], F32)
        nc.vector.reduce_sum(augsum[:], aug_row[:], axis=mybir.AxisListType.X)

        ssr = small.tile([1, 1], F32)
        nc.scalar.sqrt(ssr[:], ss[:])            # |colsum|
        ssi = small.tile([1, 1], F32)
        nc.vector.reciprocal(ssi[:], ssr[:])
        qp = small.tile([1, 1], F32)
        nc.vector.tensor_mul(qp[:], s1[:], ssi[:])      # q . a[:D]
        nc.vector.scalar_tensor_tensor(
            out=qp[:], in0=augsum[:], scalar=0.5, in1=qp[:],
            op0=mybir.AluOpType.mult, op1=mybir.AluOpType.add)
        qdiv = small.tile([1, 1], F32)
        nc.vector.tensor_scalar_mul(qdiv[:], qp[:], 1.0 / R_BIN)

        def floor_(dst_pool, src, shape, tag):
            ti = dst_pool.tile(shape, I32, tag=tag + "_i", name=tag + "_i")
            nc.vector.tensor_copy(out=ti[:], in_=src[:])
            tf = dst_pool.tile(shape, F32, tag=tag + "_f", name=tag + "_f")
            nc.vector.tensor_copy(out=tf[:], in_=ti[:])
            gt = dst_pool.tile(shape, F32, tag=tag + "_g", name=tag + "_g")
            nc.vector.tensor_tensor(
                out=gt[:], in0=tf[:], in1=src[:], op=mybir.AluOpType.is_gt)
            h = dst_pool.tile(shape, F32, tag=tag + "_h", name=tag + "_h")
            nc.vector.tensor_sub(h[:], tf[:], gt[:])
            return h

        def bucket_(dst_pool, src, shape, tag):
            # abs(mod(floor(src), TABLE_SIZE)) for values in (-TS, TS)
            h = floor_(dst_pool, src, shape, tag)
            neg = dst_pool.tile(shape, F32, tag=tag + "_n", name=tag + "_n")
            nc.vector.tensor_scalar(
                out=neg[:], in0=h[:], scalar1=0.0, scalar2=None,
                op0=mybir.AluOpType.is_lt)
            nc.vector.scalar_tensor_tensor(
                out=h[:], in0=neg[:], scalar=TABLE_SIZE, in1=h[:],
                op0=mybir.AluOpType.mult, op1=mybir.AluOpType.add)
            return h

        qb = bucket_(small, qdiv, [1, 1], "qb")
        qb_rep = small.tile([P, 1], F32)
        nc.gpsimd.partition_broadcast(qb_rep[:], qb[:])

        # scale s = U / sqrt(gmax), replicated on all partitions
        sq = small.tile([P, 1], F32)
        nc.scalar.sqrt(sq[:], gmax[:])
        sinv = small.tile([P, 1], F32)
        nc.vector.reciprocal(sinv[:], sq[:])
        s_rep = small.tile([P, 1], F32)
        nc.vector.tensor_scalar_mul(s_rep[:], sinv[:], U_SCALE)
        s2_rep = small.tile([P, 1], F32)
        nc.vector.tensor_mul(s2_rep[:], s_rep[:], s_rep[:])

        # ---- per-row bucket + mask ([P, NT]) ------------------------------
        n2 = small.tile([P, NT], F32)
        nc.vector.tensor_scalar(
            out=n2[:], in0=sumsq[:], scalar1=s2_rep[:], scalar2=None,
            op0=mybir.AluOpType.mult)
        proj = small.tile([P, NT], F32)
        nc.vector.tensor_scalar(
            out=proj[:], in0=projr[:], scalar1=s_rep[:], scalar2=None,
            op0=mybir.AluOpType.mult)
        pw = small.tile([P, NT], F32)
        nc.vector.tensor_copy(out=pw[:], in_=n2[:])
        for i in range(M_AUG):
            nc.vector.scalar_tensor_tensor(
                out=proj[:], in0=pw[:], scalar=aug_rep[:, i:i + 1], in1=proj[:],
                op0=mybir.AluOpType.mult, op1=mybir.AluOpType.add)
            if i < M_AUG - 1:
                nc.vector.tensor_mul(pw[:], pw[:], pw[:])
        pdiv = small.tile([P, NT], F32)
        nc.vector.tensor_scalar_mul(pdiv[:], proj[:], 1.0 / R_BIN)
        rb = bucket_(small, pdiv, [P, NT], "rb")
        maskf = small.tile([P, NT], F32)
        nc.vector.tensor_scalar(
            out=maskf[:], in0=rb[:], scalar1=qb_rep[:], scalar2=None,
            op0=mybir.AluOpType.is_equal)

        # ---- compaction: per-block ranks ----------------------------------
        maskb = small.tile([P, NT], BF16)
        nc.vector.tensor_copy(out=maskb[:], in_=maskf[:])
        cc_ps = psum_misc.tile([P, NT], F32, tag="misc")
        nc.tensor.matmul(cc_ps[:], lhsT=u_tri[:], rhs=maskb[:],
                         start=True, stop=True)
        cc = small.tile([P, NT], F32)
        nc.vector.tensor_copy(out=cc[:], in_=cc_ps[:])
        csrow = small.tile([1, NT], F32)
        nc.sync.dma_start(csrow[:], cc[P - 1:P, :])
        co = small.tile([1, NT], F32)
        nc.vector.memset(co[:], 0.0)
        co3 = co[0:1, :].rearrange("o (b t) -> o b t", t=TPB)
        cs3 = csrow[0:1, :].rearrange("o (b t) -> o b t", t=TPB)
        for k in range(1, TPB):
            nc.vector.tensor_add(
                co3[:, :, k:TPB], co3[:, :, k:TPB], cs3[:, :, 0:TPB - k])
        co_rep = small.tile([P, NT], F32)
        nc.gpsimd.partition_broadcast(co_rep[:], co[:])
        # in-block rank of each row (exclusive cumsum), -1 for inactive rows
        rank = small.tile([P, NT], F32)
        nc.vector.tensor_sub(rank[:], cc[:], maskf[:])
        nc.vector.tensor_add(rank[:], rank[:], co_rep[:])
        rank1 = small.tile([P, NT], F32)
        nc.vector.tensor_scalar_add(rank1[:], rank[:], 1.0)
        renc = small.tile([P, NT], F32)
        nc.vector.tensor_mul(renc[:], rank1[:], maskf[:])
        nc.vector.tensor_scalar_add(renc[:], renc[:], -1.0)

        # one-hot rank indicators per row tile: selT[p, t, k] = (renc == k)
        for t in range(NT):
            nc.vector.tensor_scalar(
                out=selT[:, t, :], in0=iok_f[:], scalar1=renc[:, t:t + 1],
                scalar2=None, op0=mybir.AluOpType.is_equal)

        # rank row view (transposed, fp16 keeps ints <= 2048 exact)
        rpad = small.tile([P, P], FP16)
        nc.vector.memset(rpad[:], -1.0)
        nc.vector.tensor_copy(out=rpad[:, 0:NT], in_=renc[:])
        rT = small.tile([P, P], FP16)
        nc.sync.dma_start(rT[:], rpad[:], transpose=True)
        rank_d = dram.tile([NT, P], FP16)
        nc.sync.dma_start(rank_d[:], rT[0:NT, :])

        # per-block compact indices: idx[s] = n of the row with rank s
        idx_d = dram.tile([NB, KL], I16)
        for j in range(NB):
            ip1 = psum_misc.tile([1, KL], F32, tag="misc", name=f"ip1_{j}")
            ip2 = psum_misc.tile([1, KL], F32, tag="misc", name=f"ip2_{j}")
            for ti in range(TPB):
                t = j * TPB + ti
                nc.tensor.matmul(ip1[:], lhsT=pvals[:], rhs=selT[:, t, :],
                                 start=(ti == 0), stop=(ti == TPB - 1))
                nc.tensor.matmul(ip2[:], lhsT=tvals[:, t:t + 1],
                                 rhs=selT[:, t, :],
                                 start=(ti == 0), stop=(ti == TPB - 1))
            ip1s = small.tile([1, KL], F32, tag="ip1s", name=f"ip1s{j}")
            nc.vector.tensor_copy(out=ip1s[:], in_=ip1[:])
            idxf = small.tile([1, KL], F32, tag="idxf", name=f"idxf{j}")
            nc.vector.scalar_tensor_tensor(
                out=idxf[:], in0=ip2[:], scalar=float(P), in1=ip1s[:],
                op0=mybir.AluOpType.mult, op1=mybir.AluOpType.add)
            idx16 = small.tile([1, KL], I16, tag="idx16", name=f"idx16_{j}")
            nc.vector.tensor_copy(out=idx16[:], in_=idxf[:])
            nc.sync.dma_start(idx_d[j:j + 1, :], idx16[:])
        idxw = const.tile([P, NB, KL // 16], I16)
        for g in range(8):
            nc.sync.dma_start(
                idxw[g * 16:(g + 1) * 16, :, :],
                idx_d[:].rearrange("j (c p) -> p j c", p=16))

        # ---- phase 2: per-block gather + compact matmul + select-scatter --
        for j in range(NB):
            wct = wctpool.tile([P, DC, KL], BF16, tag="wct", name=f"wct{j}")
            nc.gpsimd.dma_gather(
                out_ap=wct[:], in_ap=wbf_d[:], idxs_ap=idxw[:, j, :],
                num_idxs=KL, num_idxs_reg=KL, elem_size=D, transpose=True)

            # compactT[slot, b] = sum_d WT[d, idx[slot]] * xT[d, b]
            ct = ctpool.tile([P, MC, B], BF16, tag="ct", name=f"ct{j}")
            for m in range(MC):
                for bb in range(BB):
                    pm = psum_mm.tile([P, BF], F32, tag="mm",
                                      name=f"pm{j}_{m}_{bb}")
                    for dc in range(DC):
                        nc.tensor.matmul(
                            pm[:], lhsT=wct[:, dc, m * P:(m + 1) * P],
                            rhs=xT[:, dc, bb * BF:(bb + 1) * BF],
                            start=(dc == 0), stop=(dc == DC - 1))
                    nc.scalar.copy(ct[:, m, bb * BF:(bb + 1) * BF], pm[:])

            # selection matrix for this block: sel[s, n] = (rank(n) == s)
            rrow = selpool.tile([1, NFREE], FP16, tag="rrow", name=f"rr{j}")
            nc.sync.dma_start(
                rrow[:],
                rank_d[j * TPB:(j + 1) * TPB, :].rearrange("a b -> (a b)"))
            rrep = selpool.tile([P, NFREE], FP16, tag="rrep", name=f"rrep{j}")
            nc.gpsimd.partition_broadcast(rrep[:], rrow[:])
            sel = selpool.tile([P, MC, NFREE], BF16, tag="sel", name=f"sel{j}")
            for m in range(MC):
                nc.vector.tensor_scalar(
                    out=sel[:, m, :], in0=rrep[:], scalar1=iotam[m],
                    scalar2=None, op0=mybir.AluOpType.is_equal)

            # dense output block via selection matmul (exact zeros elsewhere)
            for bt in range(BT):
                po = psum_sc.tile([P, NFREE], F32, tag="sc",
                                  name=f"po{j}_{bt}")
                for m in range(MC):
                    nc.tensor.matmul(
                        po[:], lhsT=ct[:, m, bt * P:(bt + 1) * P],
                        rhs=sel[:, m, :],
                        start=(m == 0), stop=(m == MC - 1))
                ob = outpool.tile([P, NFREE], F32, tag="ob",
                                  name=f"ob{j}_{bt}")
                nc.scalar.copy(ob[:], po[:])
                nc.scalar.dma_start(
                    out_h.ap()[bt * P:(bt + 1) * P,
                               j * NFREE:(j + 1) * NFREE], ob[:])

    nc.compile()
    return nc


_CACHE = {}
LAST_RESULTS = None


def _get_program(B, D, Nl, n_cores):
    key = (B, D, Nl, n_cores)
    if key not in _CACHE:
        _CACHE[key] = build_program(B, D, Nl, n_cores)
    return _CACHE[key]


def kernel(x, W, a, mode=None, trace=False):
    global LAST_RESULTS
    _install_ntff_shim()
    from concourse.bass_utils import run_bass_kernel_spmd

    x = np.ascontiguousarray(np.asarray(x, dtype=np.float32))
    W = np.ascontiguousarray(np.asarray(W, dtype=np.float32))
    a = np.ascontiguousarray(np.asarray(a, dtype=np.float32))
    B, D = x.shape
    N = W.shape[0]
    n_cores = 8
    Nl = N // n_cores
    nc = _get_program(B, D, Nl, n_cores)

    in_maps = [
        {"x": x, "W": W[i * Nl:(i + 1) * Nl], "a": a} for i in range(n_cores)
    ]
    res = run_bass_kernel_spmd(
        nc, in_maps, core_ids=list(range(n_cores)), trace=trace)
    LAST_RESULTS = res
    out = np.concatenate([res.results[i]["out"] for i in range(n_cores)], axis=1)
    return out.astype(np.float32)
